# revision 25
# baseline (speedup 1.0000x reference)
"""Trainium2 Bass kernel for nn_KC_Avg_Embedding (multi-hot averaged embedding).

Computes, for multi-hot indicator vectors x[b,s,:] over a vocabulary of 1024:
    out[b,s,:] = (x[b,s,:] @ E) / max(sum(x[b,s,:]), 1)

Strategy (data-parallel over 8 NeuronCores, batch-sharded; memory-regime):
  - Each core gets rows = (B/8)*S = 3200 rows. The host uploads x already
    TRANSPOSED, swizzled into the exact SBUF tile layout [p, chunk, row] per
    DMA group, and cast to fp8e4m3 (x is 0/1 so fp8 is exact): 4x less HBM
    traffic than fp32, no on-device transposes, and every DMA descriptor is
    a contiguous multi-KB run per partition.
  - E is uploaded as fp16 [p, chunk, 129] with a ones column appended on
    host; the ones column makes the row-count (averaging denominator) fall
    out of the same matmuls.
  - Per 128-row tile: 8 accumulating matmuls (one per 128-wide vocab chunk),
    lhsT = x^T chunk (fp8 stationary -> fast weight load), rhs = E_aug fp16
    (moving, N=129), fp32 PSUM -> [128 rows, 129] = [x@E | count].
  - Epilogue: ACT stages PSUM->SBUF (single sync-wait), DVE computes
    r = 1/max(count,1) and scales, writing fp16; scalar-engine HWDGE ring
    DMAs the fp16 output out. Host upcasts to fp32.
  - Row-tile groups are sized [2,3,5,5,5,3,2]: small first group so matmuls
    start as early as possible, small last group so the drain tail is short.
  - Dummy matmuls on zeroed tiles warm the PE HAM clock gate during the
    initial DMA; HBM per core: 3.28 MB (x) + 0.26 MB (E) + 0.82 MB (y).
"""

import sys
from contextlib import ExitStack

import numpy as np
import ml_dtypes

for _p in ("/opt/trn_rl_repo",):
    if _p not in sys.path:
        sys.path.insert(0, _p)

import concourse.bass as bass
import concourse.mybir as mybir
import concourse.tile as tile

from concourse.vector_clock import ScopedClock


class _SplitDrainTC(tile.TileContext):
    """TileContext whose kernel-tail drain splits its semaphore waits across
    single-wait carrier nops — this walrus build enforces a small
    per-instruction sync-wait limit that the stock all-lane drain exceeds."""

    def _drain_and_barrier(self, tick_clock, wait_clock):
        drain_inst = self.nc.sync.drain()
        wait_clock.add_sem_waits(
            drain_inst.ins, ScopedClock({None: tick_clock.global_clock})
        )
        si = drain_inst.ins.sync_info
        if si is not None and si.on_wait is not None and len(si.on_wait) > 1:
            waits = list(si.on_wait)
            del si.on_wait[1:]
            for w in waits[1:]:
                nop = self.nc.sync.nop(nofuse=True, hint="drain_wait_split")
                nsi = nop.ins.sync_info
                if nsi is None:
                    nop.ins.sync_info = mybir.SyncInfo(on_update=[], on_wait=[w])
                else:
                    nsi.on_wait.append(w)
        self.nc.all_engine_barrier()
        assert self.sems is not None
        popped = self.nc._tile_sem_poison_stack.pop()
        assert popped is self._sem_poison
        self.nc.clear_and_free_semaphores(list(self.sems.allocated().values()))
        self.nc.all_engine_barrier()


B, S, V, D = 128, 200, 1024, 128
NCORES = 8
P = 128
PER_CORE_B = B // NCORES          # 16
ROWS = PER_CORE_B * S             # 3200 rows per core
NCH = V // P                      # 8 vocab chunks
NE = D + 1                        # 128 emb cols + 1 count col
GSIZES = [4, 4, 5, 5, 5, 2]       # row tiles per DMA group (sum = 25)
NG = len(GSIZES)
NCH_H = NCH // 2                  # E loads as two half-tiles of 4 chunks
WARMUP_MM = 30                    # dummy matmuls to warm the PE HAM clock gate

FP8 = mybir.dt.float8e4
F16 = mybir.dt.float16
F32 = mybir.dt.float32
NP_FP8 = ml_dtypes.float8_e4m3
FP8_ONE = 0x38                    # bit pattern of 1.0 in fp8e4m3

assert sum(GSIZES) * P == ROWS


def build_kernel():
    nc = bass.Bass()
    # x^T per group, pre-swizzled on host to [p, chunk, row]: fp8, 0/1 exact
    xs = [
        nc.declare_dram_parameter(f"x{g}", [P, NCH, GSIZES[g] * P], FP8,
                                  isOutput=False)
        for g in range(NG)
    ]
    # E augmented with ones column, fp16, host-swizzled to [p, chunk, col]
    emb = nc.declare_dram_parameter("emb", [P, NCH, NE], F16, isOutput=False)
    y = nc.declare_dram_parameter("y", [ROWS, D], F16, isOutput=True)

    # First-wave data (first E half + x0) is DMA'd BEFORE the tile-context
    # entry barrier, into raw SBUF tensors: the bytes stream ~1.5us earlier,
    # while register init and the tile-context barrier are still running.
    # Both transfers go on the SAME HWDGE ring in order [emb_a, x0]: each
    # SDMA engine consumes a ring's descriptors in order, so x0's
    # completion semaphore implies emb_a has fully landed — one manual
    # semaphore covers both raw tensors.
    xb0_raw = nc.alloc_sbuf_tensor("xb0_raw", [P, NCH, GSIZES[0] * P], FP8)
    rhsa_raw = nc.alloc_sbuf_tensor("rhsa_raw", [P, NCH_H, NE], F16)
    sem_x0 = nc.alloc_semaphore("sem_x0")
    sem_ea = nc.alloc_semaphore("sem_ea")
    # walrus requires every DGE DMA to carry sync info; nothing waits on
    # sem_ea — ring FIFO makes sem_x0 imply emb_a completion
    nc.scalar.dma_start(rhsa_raw.ap(), emb[:, 0:NCH_H, :]).then_inc(sem_ea, 16)
    nc.scalar.dma_start(xb0_raw.ap(), xs[0][:]).then_inc(sem_x0, 16)

    with _SplitDrainTC(nc) as tc, ExitStack() as ctx:
        const = ctx.enter_context(tc.tile_pool(name="const", bufs=1))
        xb_pool = ctx.enter_context(tc.tile_pool(name="xb", bufs=NG))
        out_pool = ctx.enter_context(tc.tile_pool(name="out", bufs=NG))
        stage_pool = ctx.enter_context(tc.tile_pool(name="stage", bufs=NG))
        small = ctx.enter_context(tc.tile_pool(name="small", bufs=NG))
        psum_w = ctx.enter_context(tc.tile_pool(name="psum_w", bufs=1, space="PSUM"))
        psum_o = ctx.enter_context(tc.tile_pool(name="psum_o", bufs=4, space="PSUM"))

        # Input DMAs alternate between the two HWDGE rings (Sync + Scalar):
        # each DMA_DIRECT2D occupies its sequencer ~0.65us generating
        # descriptors, so one ring would serialize ~5us of issue latency.
        # x0 goes first (smallest group -> earliest first matmul), emb in
        # parallel on the other ring.
        xb_tiles = [None]
        for g in range(1, NG):
            xb = xb_pool.tile([P, NCH, GSIZES[g] * P], FP8)
            xb_tiles.append(xb)
        rhs_b = const.tile([P, NCH_H, NE], F16)
        # ring A (sync): rhs_b, x1, x3, x5;
        # ring B (scalar): [rhs_a, x0 pre-TC], x2, x4
        nc.sync.dma_start(rhs_b[:], emb[:, NCH_H:NCH, :])
        for g in range(1, NG):
            eng = nc.sync if g % 2 == 1 else nc.scalar
            eng.dma_start(xb_tiles[g][:], xs[g][:])

        # Dummy matmuls on zeroed tiles: warm the PE clock gate (HAM) and
        # keep it busy until the first x group lands; memsets on DVE (fast
        # launch, idle early).
        wz = const.tile([P, P], FP8)
        ez = const.tile([P, NE], F16)
        nc.vector.memset(wz[:], 0.0)
        nc.vector.memset(ez[:], 0.0)
        pw = psum_w.tile([P, NE], F32)
        for i in range(WARMUP_MM):
            nc.tensor.matmul(pw[:], wz[:], ez[:], start=True, stop=True)

        # y rows are tile-major: row = t*128 + p
        yt = y.rearrange("(t p) d -> p t d", p=P)

        rhsa_ap = rhsa_raw.ap()
        xb0_ap = xb0_raw.ap()
        g0_c0_mms = []
        t0 = 0
        for g, sz in enumerate(GSIZES):
            # ACT stages each PSUM tile to SBUF (single sync-wait on the PE);
            # DVE then does the whole group's normalization from SBUF.
            stage = stage_pool.tile([P, sz, NE], F32)
            for f in range(sz):
                po = psum_o.tile([P, NE], F32)
                for c in range(NCH):
                    xa = xb0_ap if g == 0 else xb_tiles[g]
                    rh = rhsa_ap if c < NCH_H else rhs_b[:]
                    mm = nc.tensor.matmul(po[:], xa[:, c, f * P:(f + 1) * P],
                                          rh[:, c % NCH_H, :],
                                          start=(c == 0), stop=(c == NCH - 1))
                    if g == 0 and c == 0:
                        g0_c0_mms.append(mm)
                nc.scalar.copy(stage[:, f, :], po[:])
            r = small.tile([P, sz], F32)
            nc.vector.tensor_scalar_max(r[:], stage[:, :, D], 1.0)
            nc.vector.reciprocal(r[:], r[:])
            out_sb = out_pool.tile([P, sz, D], F16)
            for f in range(sz):
                nc.vector.tensor_scalar_mul(out_sb[:, f, :], stage[:, f, 0:D],
                                            r[:, f:f + 1])
            # SWDGE tolerates the multi-sem-wait this join needs (HWDGE
            # DMA instructions are limited to one sync-wait), and DMA
            # completion is receipt-dominated (~2us) on either path
            nc.gpsimd.dma_start(yt[:, t0:t0 + sz, :], out_sb[:])
            t0 += sz

    # The waits for the pre-TC DMAs are attached AFTER the tile scheduler
    # runs (it cannot model semaphores incremented outside its world and
    # would report a deadlock). Each group-0 tile's first matmul carries
    # both waits: those are the only possible first readers of the raw
    # tensors — group 1's matmuls are gated on the x1 DMA, which is FIFO
    # behind the rhs_a DMA on the same ring, and every later tile is
    # ordered behind group 0 through the PSUM-slot reuse chain (4 slots
    # < 7 tiles in groups 0-1).
    # Post-scheduler manual waits (the tile scheduler cannot model
    # semaphores incremented outside its world): each group-0 tile's first
    # matmul waits for the pre-TC x0 DMA (which by ring FIFO implies the
    # rhs_a DMA too). Later tiles are ordered behind these through PSUM
    # slot reuse (bufs=4 < group-0's 4 tiles + anything after).
    for mm in g0_c0_mms:
        mm.wait_op(sem_x0, 16, "sem-ge", check=False)

    # clear the manual semaphore so a second execution of the same NEFF
    # starts from zero
    nc.clear_and_free_semaphores([sem_x0, sem_ea])
    return nc


_cached_nc = None


def make_in_maps(batch_vectors, embedding_matrix):
    """Host-side prep: shard + transpose + swizzle + cast. Layout/dtype only."""
    x = np.asarray(batch_vectors, dtype=np.float32).reshape(B, S, V)
    e = np.asarray(embedding_matrix, dtype=np.float32)
    e_aug = np.empty((V, NE), dtype=np.float16)
    e_aug[:, 0:D] = e.astype(np.float16)
    e_aug[:, D] = np.float16(1.0)
    # [V, NE] -> [p, chunk, NE]
    e_dev = np.ascontiguousarray(e_aug.reshape(NCH, P, NE).transpose(1, 0, 2))

    # 0/1 -> fp8 bit pattern, then pure reshape/transpose per group
    xb = (x != 0).astype(np.uint8) * np.uint8(FP8_ONE)
    in_maps = []
    for i in range(NCORES):
        shard = xb[i * PER_CORE_B:(i + 1) * PER_CORE_B].reshape(ROWS, V)
        m = {"emb": e_dev}
        t0 = 0
        for g, sz in enumerate(GSIZES):
            blk = shard[t0 * P:(t0 + sz) * P, :].T        # [V, sz*P]
            blk = blk.reshape(NCH, P, sz * P).transpose(1, 0, 2)
            m[f"x{g}"] = np.ascontiguousarray(blk).view(NP_FP8)
            t0 += sz
        in_maps.append(m)
    return in_maps


def kernel(**inputs):
    global _cached_nc
    from concourse.bass_utils import run_bass_kernel_spmd

    if _cached_nc is None:
        _cached_nc = build_kernel()

    in_maps = make_in_maps(inputs["batch_vectors"], inputs["embedding_matrix"])
    res = run_bass_kernel_spmd(_cached_nc, in_maps, core_ids=list(range(NCORES)))
    out = np.concatenate(
        [
            res.results[i]["y"].astype(np.float32).reshape(PER_CORE_B, S, D)
            for i in range(NCORES)
        ],
        axis=0,
    )
    return out


# revision 26
# speedup vs baseline: 1.0409x; 1.0409x over previous
"""Trainium2 Bass kernel for nn_KC_Avg_Embedding (multi-hot averaged embedding).

Computes, for multi-hot indicator vectors x[b,s,:] over a vocabulary of 1024:
    out[b,s,:] = (x[b,s,:] @ E) / max(sum(x[b,s,:]), 1)

Strategy (data-parallel over 8 NeuronCores, batch-sharded; memory-regime):
  - Each core gets rows = (B/8)*S = 3200 rows. The host uploads x already
    TRANSPOSED, swizzled into the exact SBUF tile layout [p, chunk, row] per
    DMA group, and cast to fp8e4m3 (x is 0/1 so fp8 is exact): 4x less HBM
    traffic than fp32, no on-device transposes, and every DMA descriptor is
    a contiguous multi-KB run per partition.
  - E is uploaded as fp16 [p, chunk, 129] with a ones column appended on
    host; the ones column makes the row-count (averaging denominator) fall
    out of the same matmuls.
  - Per 128-row tile: 8 accumulating matmuls (one per 128-wide vocab chunk),
    lhsT = x^T chunk (fp8 stationary -> fast weight load), rhs = E_aug fp16
    (moving, N=129), fp32 PSUM -> [128 rows, 129] = [x@E | count].
  - Epilogue: ACT stages PSUM->SBUF (single sync-wait), DVE computes
    r = 1/max(count,1) and scales, writing fp16; scalar-engine HWDGE ring
    DMAs the fp16 output out. Host upcasts to fp32.
  - Row-tile groups are sized [2,3,5,5,5,3,2]: small first group so matmuls
    start as early as possible, small last group so the drain tail is short.
  - Dummy matmuls on zeroed tiles warm the PE HAM clock gate during the
    initial DMA; HBM per core: 3.28 MB (x) + 0.26 MB (E) + 0.82 MB (y).
"""

import sys
from contextlib import ExitStack

import numpy as np
import ml_dtypes

for _p in ("/opt/trn_rl_repo",):
    if _p not in sys.path:
        sys.path.insert(0, _p)

import concourse.bass as bass
import concourse.mybir as mybir
import concourse.tile as tile

from concourse.vector_clock import ScopedClock


class _SplitDrainTC(tile.TileContext):
    """TileContext whose kernel-tail drain splits its semaphore waits across
    single-wait carrier nops — this walrus build enforces a small
    per-instruction sync-wait limit that the stock all-lane drain exceeds."""

    def _drain_and_barrier(self, tick_clock, wait_clock):
        drain_inst = self.nc.sync.drain()
        wait_clock.add_sem_waits(
            drain_inst.ins, ScopedClock({None: tick_clock.global_clock})
        )
        si = drain_inst.ins.sync_info
        if si is not None and si.on_wait is not None and len(si.on_wait) > 1:
            waits = list(si.on_wait)
            del si.on_wait[1:]
            for w in waits[1:]:
                nop = self.nc.sync.nop(nofuse=True, hint="drain_wait_split")
                nsi = nop.ins.sync_info
                if nsi is None:
                    nop.ins.sync_info = mybir.SyncInfo(on_update=[], on_wait=[w])
                else:
                    nsi.on_wait.append(w)
        self.nc.all_engine_barrier()
        assert self.sems is not None
        popped = self.nc._tile_sem_poison_stack.pop()
        assert popped is self._sem_poison
        self.nc.clear_and_free_semaphores(list(self.sems.allocated().values()))
        self.nc.all_engine_barrier()


B, S, V, D = 128, 200, 1024, 128
NCORES = 8
P = 128
PER_CORE_B = B // NCORES          # 16
ROWS = PER_CORE_B * S             # 3200 rows per core
NCH = V // P                      # 8 vocab chunks
NE = D + 1                        # 128 emb cols + 1 count col
GSIZES = [4, 3, 5, 5, 5, 3]       # row tiles per DMA group (sum = 25)
NG = len(GSIZES)
NCH_H = NCH // 2                  # E loads as two half-tiles of 4 chunks
WARMUP_MM = 26                    # dummy matmuls to warm the PE HAM clock gate

FP8 = mybir.dt.float8e4
F16 = mybir.dt.float16
F32 = mybir.dt.float32
NP_FP8 = ml_dtypes.float8_e4m3
FP8_ONE = 0x38                    # bit pattern of 1.0 in fp8e4m3

assert sum(GSIZES) * P == ROWS


def build_kernel():
    nc = bass.Bass()
    # x^T per group, pre-swizzled on host to [p, chunk, row]: fp8, 0/1 exact
    xs = [
        nc.declare_dram_parameter(f"x{g}", [P, NCH, GSIZES[g] * P], FP8,
                                  isOutput=False)
        for g in range(NG)
    ]
    # E augmented with ones column, fp16, host-swizzled to [p, chunk, col]
    emb = nc.declare_dram_parameter("emb", [P, NCH, NE], F16, isOutput=False)
    y = nc.declare_dram_parameter("y", [ROWS, D], F16, isOutput=True)

    # The first E half is DMA'd BEFORE the tile-context entry barrier into
    # a raw SBUF tensor: its bytes stream ~1.5us early, while register
    # init and the tile-context barrier are still running. Each tile's
    # chunk loop is rotated to start on the TC-tracked second half (so
    # normal tile tracking gates the x data), and each group-0 tile's
    # first rhs_a-reading matmul carries a manual semaphore wait; all
    # later tiles are ordered behind group 0 through PSUM-slot reuse.
    rhsa_raw = nc.alloc_sbuf_tensor("rhsa_raw", [P, NCH_H, NE], F16)
    sem_ea = nc.alloc_semaphore("sem_ea")
    nc.scalar.dma_start(rhsa_raw.ap(), emb[:, 0:NCH_H, :]).then_inc(sem_ea, 16)

    with _SplitDrainTC(nc) as tc, ExitStack() as ctx:
        const = ctx.enter_context(tc.tile_pool(name="const", bufs=1))
        xb_pool = ctx.enter_context(tc.tile_pool(name="xb", bufs=NG))
        out_pool = ctx.enter_context(tc.tile_pool(name="out", bufs=NG))
        stage_pool = ctx.enter_context(tc.tile_pool(name="stage", bufs=NG))
        small = ctx.enter_context(tc.tile_pool(name="small", bufs=NG))
        psum_w = ctx.enter_context(tc.tile_pool(name="psum_w", bufs=1, space="PSUM"))
        psum_o = ctx.enter_context(tc.tile_pool(name="psum_o", bufs=4, space="PSUM"))

        # Input DMAs alternate between the two HWDGE rings (Sync + Scalar):
        # each DMA_DIRECT2D occupies its sequencer ~0.65us generating
        # descriptors, so one ring would serialize ~5us of issue latency.
        # x0 goes first (smallest group -> earliest first matmul), emb in
        # parallel on the other ring.
        xb_tiles = []
        for g, sz in enumerate(GSIZES):
            xb = xb_pool.tile([P, NCH, sz * P], FP8)
            xb_tiles.append(xb)
        rhs_b = const.tile([P, NCH_H, NE], F16)
        # ring A (sync): x0, x2, x4; ring B (scalar): [rhs_a pre-TC],
        # rhs_b, x1, x3, x5 — groups land roughly in consumption order
        nc.sync.dma_start(xb_tiles[0][:], xs[0][:])
        nc.scalar.dma_start(rhs_b[:], emb[:, NCH_H:NCH, :])
        for g in range(1, NG):
            eng = nc.scalar if g % 2 == 1 else nc.sync
            eng.dma_start(xb_tiles[g][:], xs[g][:])

        # Dummy matmuls on zeroed tiles: warm the PE clock gate (HAM) and
        # keep it busy until the first x group lands; memsets on DVE (fast
        # launch, idle early).
        wz = const.tile([P, P], FP8)
        ez = const.tile([P, NE], F16)
        nc.vector.memset(wz[:], 0.0)
        nc.vector.memset(ez[:], 0.0)
        pw = psum_w.tile([P, NE], F32)
        for _ in range(WARMUP_MM):
            nc.tensor.matmul(pw[:], wz[:], ez[:], start=True, stop=True)

        # y rows are tile-major: row = t*128 + p
        yt = y.rearrange("(t p) d -> p t d", p=P)

        rhsa_ap = rhsa_raw.ap()
        g0_gate_mms = []
        t0 = 0
        for g, sz in enumerate(GSIZES):
            xb = xb_tiles[g]
            # ACT stages each PSUM tile to SBUF (single sync-wait on the PE);
            # DVE then does the whole group's normalization from SBUF.
            stage = stage_pool.tile([P, sz, NE], F32)
            for f in range(sz):
                po = psum_o.tile([P, NE], F32)
                for cc in range(NCH):
                    c = (cc + NCH_H) % NCH   # chunks 4..7 (tracked) first
                    rh = (rhs_b[:, c - NCH_H, :] if c >= NCH_H
                          else rhsa_ap[:, c, :])
                    mm = nc.tensor.matmul(po[:], xb[:, c, f * P:(f + 1) * P],
                                          rh,
                                          start=(cc == 0), stop=(cc == NCH - 1))
                    if g == 0 and cc == NCH_H:
                        g0_gate_mms.append(mm)
                nc.scalar.copy(stage[:, f, :], po[:])
            r = small.tile([P, sz], F32)
            nc.vector.tensor_scalar_max(r[:], stage[:, :, D], 1.0)
            nc.vector.reciprocal(r[:], r[:])
            out_sb = out_pool.tile([P, sz, D], F16)
            for f in range(sz):
                nc.vector.tensor_scalar_mul(out_sb[:, f, :], stage[:, f, 0:D],
                                            r[:, f:f + 1])
            # SWDGE tolerates the multi-sem-wait this join needs (HWDGE
            # DMA instructions are limited to one sync-wait), and DMA
            # completion is receipt-dominated (~2us) on either path
            nc.gpsimd.dma_start(yt[:, t0:t0 + sz, :], out_sb[:])
            t0 += sz

    # attach the raw-rhs_a gate after the tile scheduler runs (it cannot
    # model semaphores incremented outside its world)
    for mm in g0_gate_mms:
        mm.wait_op(sem_ea, 16, "sem-ge", check=False)
    # clear the manual semaphore so a second execution starts from zero
    nc.clear_and_free_semaphores([sem_ea])
    return nc


_cached_nc = None


def make_in_maps(batch_vectors, embedding_matrix):
    """Host-side prep: shard + transpose + swizzle + cast. Layout/dtype only."""
    x = np.asarray(batch_vectors, dtype=np.float32).reshape(B, S, V)
    e = np.asarray(embedding_matrix, dtype=np.float32)
    e_aug = np.empty((V, NE), dtype=np.float16)
    e_aug[:, 0:D] = e.astype(np.float16)
    e_aug[:, D] = np.float16(1.0)
    # [V, NE] -> [p, chunk, NE]
    e_dev = np.ascontiguousarray(e_aug.reshape(NCH, P, NE).transpose(1, 0, 2))

    # 0/1 -> fp8 bit pattern, then pure reshape/transpose per group
    xb = (x != 0).astype(np.uint8) * np.uint8(FP8_ONE)
    in_maps = []
    for i in range(NCORES):
        shard = xb[i * PER_CORE_B:(i + 1) * PER_CORE_B].reshape(ROWS, V)
        m = {"emb": e_dev}
        t0 = 0
        for g, sz in enumerate(GSIZES):
            blk = shard[t0 * P:(t0 + sz) * P, :].T        # [V, sz*P]
            blk = blk.reshape(NCH, P, sz * P).transpose(1, 0, 2)
            m[f"x{g}"] = np.ascontiguousarray(blk).view(NP_FP8)
            t0 += sz
        in_maps.append(m)
    return in_maps


def kernel(**inputs):
    global _cached_nc
    from concourse.bass_utils import run_bass_kernel_spmd

    if _cached_nc is None:
        _cached_nc = build_kernel()

    in_maps = make_in_maps(inputs["batch_vectors"], inputs["embedding_matrix"])
    res = run_bass_kernel_spmd(_cached_nc, in_maps, core_ids=list(range(NCORES)))
    out = np.concatenate(
        [
            res.results[i]["y"].astype(np.float32).reshape(PER_CORE_B, S, D)
            for i in range(NCORES)
        ],
        axis=0,
    )
    return out


# revision 27
# speedup vs baseline: 1.1577x; 1.1122x over previous
"""Trainium2 Bass kernel for nn_KC_Avg_Embedding (multi-hot averaged embedding).

Computes, for multi-hot indicator vectors x[b,s,:] over a vocabulary of 1024:
    out[b,s,:] = (x[b,s,:] @ E) / max(sum(x[b,s,:]), 1)

Strategy (data-parallel over 8 NeuronCores, batch-sharded; memory-regime):
  - Each core gets rows = (B/8)*S = 3200 rows. The host uploads x already
    TRANSPOSED, swizzled into the exact SBUF tile layout [p, chunk, row] per
    DMA group, and cast to fp8e4m3 (x is 0/1 so fp8 is exact): 4x less HBM
    traffic than fp32, no on-device transposes, and every DMA descriptor is
    a contiguous multi-KB run per partition.
  - E is uploaded as fp16 [p, chunk, 129] with a ones column appended on
    host; the ones column makes the row-count (averaging denominator) fall
    out of the same matmuls.
  - Per 128-row tile: 8 accumulating matmuls (one per 128-wide vocab chunk),
    lhsT = x^T chunk (fp8 stationary -> fast weight load), rhs = E_aug fp16
    (moving, N=129), fp32 PSUM -> [128 rows, 129] = [x@E | count].
  - Epilogue: ACT stages PSUM->SBUF (single sync-wait), DVE computes
    r = 1/max(count,1) and scales, writing fp16; scalar-engine HWDGE ring
    DMAs the fp16 output out. Host upcasts to fp32.
  - Row-tile groups are sized [2,3,5,5,5,3,2]: small first group so matmuls
    start as early as possible, small last group so the drain tail is short.
  - Dummy matmuls on zeroed tiles warm the PE HAM clock gate during the
    initial DMA; HBM per core: 3.28 MB (x) + 0.26 MB (E) + 0.82 MB (y).
"""

import sys
from contextlib import ExitStack

import numpy as np
import ml_dtypes

for _p in ("/opt/trn_rl_repo",):
    if _p not in sys.path:
        sys.path.insert(0, _p)

import concourse.bass as bass
import concourse.mybir as mybir
import concourse.tile as tile

from concourse.vector_clock import ScopedClock


class _SplitDrainTC(tile.TileContext):
    """TileContext whose kernel-tail drain splits its semaphore waits across
    single-wait carrier nops — this walrus build enforces a small
    per-instruction sync-wait limit that the stock all-lane drain exceeds."""

    def _drain_and_barrier(self, tick_clock, wait_clock):
        drain_inst = self.nc.sync.drain()
        wait_clock.add_sem_waits(
            drain_inst.ins, ScopedClock({None: tick_clock.global_clock})
        )
        si = drain_inst.ins.sync_info
        if si is not None and si.on_wait is not None and len(si.on_wait) > 1:
            waits = list(si.on_wait)
            del si.on_wait[1:]
            for w in waits[1:]:
                nop = self.nc.sync.nop(nofuse=True, hint="drain_wait_split")
                nsi = nop.ins.sync_info
                if nsi is None:
                    nop.ins.sync_info = mybir.SyncInfo(on_update=[], on_wait=[w])
                else:
                    nsi.on_wait.append(w)
        self.nc.all_engine_barrier()
        assert self.sems is not None
        popped = self.nc._tile_sem_poison_stack.pop()
        assert popped is self._sem_poison
        self.nc.clear_and_free_semaphores(list(self.sems.allocated().values()))
        self.nc.all_engine_barrier()


B, S, V, D = 128, 200, 1024, 128
NCORES = 8
P = 128
PER_CORE_B = B // NCORES          # 16
ROWS = PER_CORE_B * S             # 3200 rows per core
NCH = V // P                      # 8 vocab chunks
NE = D + 1                        # 128 emb cols + 1 count col
GSIZES = [4, 4, 5, 5, 5, 2]       # row tiles per DMA group (sum = 25)
NG = len(GSIZES)
NCH_H = NCH // 2                  # E loads as two half-tiles of 4 chunks
WARMUP_MM = 45                    # dummy matmuls to warm the PE HAM clock gate

FP8 = mybir.dt.float8e4
F16 = mybir.dt.float16
F32 = mybir.dt.float32
NP_FP8 = ml_dtypes.float8_e4m3
FP8_ONE = 0x38                    # bit pattern of 1.0 in fp8e4m3

assert sum(GSIZES) * P == ROWS


def build_kernel():
    nc = bass.Bass()
    # x^T per group, pre-swizzled on host to [p, chunk, row]: fp8, 0/1 exact
    xs = [
        nc.declare_dram_parameter(f"x{g}", [P, NCH, GSIZES[g] * P], FP8,
                                  isOutput=False)
        for g in range(NG)
    ]
    # E augmented with ones column, fp16, host-swizzled to [p, chunk, col]
    emb = nc.declare_dram_parameter("emb", [P, NCH, NE], F16, isOutput=False)
    y = nc.declare_dram_parameter("y", [ROWS, D], F16, isOutput=True)

    with _SplitDrainTC(nc) as tc, ExitStack() as ctx:
        const = ctx.enter_context(tc.tile_pool(name="const", bufs=1))
        xb_pool = ctx.enter_context(tc.tile_pool(name="xb", bufs=NG))
        out_pool = ctx.enter_context(tc.tile_pool(name="out", bufs=NG))
        stage_pool = ctx.enter_context(tc.tile_pool(name="stage", bufs=NG))
        small = ctx.enter_context(tc.tile_pool(name="small", bufs=NG))
        psum_w = ctx.enter_context(tc.tile_pool(name="psum_w", bufs=1, space="PSUM"))
        psum_o = ctx.enter_context(tc.tile_pool(name="psum_o", bufs=4, space="PSUM"))

        # Input DMAs alternate between the two HWDGE rings (Sync + Scalar):
        # each DMA_DIRECT2D occupies its sequencer ~0.65us generating
        # descriptors, so one ring would serialize ~5us of issue latency.
        # x0 goes first (smallest group -> earliest first matmul), emb in
        # parallel on the other ring.
        xb_tiles = []
        for g, sz in enumerate(GSIZES):
            xb = xb_pool.tile([P, NCH, sz * P], FP8)
            xb_tiles.append(xb)
        # E as two independent half-tiles: the first matmuls only gate on
        # x0 + the first E half (~260KB across both rings), so the PE
        # starts ~0.8us after first bytes instead of ~1.6us.
        rhs_a = const.tile([P, NCH_H, NE], F16)
        rhs_b = const.tile([P, NCH_H, NE], F16)
        # Small groups, strictly alternating across the two HWDGE rings so
        # each group's arrival tracks the aggregate DMA pace. Input DMAs
        # carry at most the lane-reuse chain wait (1), so more than 8
        # HWDGE input DMAs is fine; outputs (which need their one wait for
        # data-ready) stay on SWDGE, at most 8 of them.
        # ring A (sync): x0, rhs_b, x2, x4; ring B (scalar): rhs_a, x1, x3, x5
        nc.sync.dma_start(xb_tiles[0][:], xs[0][:])
        nc.scalar.dma_start(rhs_a[:], emb[:, 0:NCH_H, :])
        nc.sync.dma_start(rhs_b[:], emb[:, NCH_H:NCH, :])
        for g in range(1, NG):
            eng = nc.scalar if g % 2 == 1 else nc.sync
            eng.dma_start(xb_tiles[g][:], xs[g][:])

        # Dummy matmuls on zeroed tiles: warm the PE clock gate (HAM) and
        # keep it busy until the first x group lands; memsets on DVE (fast
        # launch, idle early).
        wz = const.tile([P, P], FP8)
        ez = const.tile([P, NE], F16)
        nc.vector.memset(wz[:], 0.0)
        nc.vector.memset(ez[:], 0.0)
        pw = psum_w.tile([P, NE], F32)
        for _ in range(WARMUP_MM):
            nc.tensor.matmul(pw[:], wz[:], ez[:], start=True, stop=True)

        # y rows are tile-major: row = t*128 + p
        yt = y.rearrange("(t p) d -> p t d", p=P)

        t0 = 0
        for g, sz in enumerate(GSIZES):
            xb = xb_tiles[g]
            # ACT stages each PSUM tile to SBUF (single sync-wait on the PE);
            # DVE then does the whole group's normalization from SBUF.
            stage = stage_pool.tile([P, sz, NE], F32)
            for f in range(sz):
                po = psum_o.tile([P, NE], F32)
                for c in range(NCH):
                    rh = rhs_a if c < NCH_H else rhs_b
                    nc.tensor.matmul(po[:], xb[:, c, f * P:(f + 1) * P],
                                     rh[:, c % NCH_H, :],
                                     start=(c == 0), stop=(c == NCH - 1))
                nc.scalar.copy(stage[:, f, :], po[:])
            r = small.tile([P, sz], F32)
            nc.vector.tensor_scalar_max(r[:], stage[:, :, D], 1.0)
            nc.vector.reciprocal(r[:], r[:])
            out_sb = out_pool.tile([P, sz, D], F16)
            for f in range(sz):
                nc.vector.tensor_scalar_mul(out_sb[:, f, :], stage[:, f, 0:D],
                                            r[:, f:f + 1])
            # SWDGE tolerates the multi-sem-wait this join needs (HWDGE
            # DMA instructions are limited to one sync-wait), and DMA
            # completion is receipt-dominated (~2us) on either path
            nc.gpsimd.dma_start(yt[:, t0:t0 + sz, :], out_sb[:])
            t0 += sz

    return nc


_cached_nc = None


def make_in_maps(batch_vectors, embedding_matrix):
    """Host-side prep: shard + transpose + swizzle + cast. Layout/dtype only."""
    x = np.asarray(batch_vectors, dtype=np.float32).reshape(B, S, V)
    e = np.asarray(embedding_matrix, dtype=np.float32)
    e_aug = np.empty((V, NE), dtype=np.float16)
    e_aug[:, 0:D] = e.astype(np.float16)
    e_aug[:, D] = np.float16(1.0)
    # [V, NE] -> [p, chunk, NE]
    e_dev = np.ascontiguousarray(e_aug.reshape(NCH, P, NE).transpose(1, 0, 2))

    # 0/1 -> fp8 bit pattern, then pure reshape/transpose per group
    xb = (x != 0).astype(np.uint8) * np.uint8(FP8_ONE)
    in_maps = []
    for i in range(NCORES):
        shard = xb[i * PER_CORE_B:(i + 1) * PER_CORE_B].reshape(ROWS, V)
        m = {"emb": e_dev}
        t0 = 0
        for g, sz in enumerate(GSIZES):
            blk = shard[t0 * P:(t0 + sz) * P, :].T        # [V, sz*P]
            blk = blk.reshape(NCH, P, sz * P).transpose(1, 0, 2)
            m[f"x{g}"] = np.ascontiguousarray(blk).view(NP_FP8)
            t0 += sz
        in_maps.append(m)
    return in_maps


def kernel(**inputs):
    global _cached_nc
    from concourse.bass_utils import run_bass_kernel_spmd

    if _cached_nc is None:
        _cached_nc = build_kernel()

    in_maps = make_in_maps(inputs["batch_vectors"], inputs["embedding_matrix"])
    res = run_bass_kernel_spmd(_cached_nc, in_maps, core_ids=list(range(NCORES)))
    out = np.concatenate(
        [
            res.results[i]["y"].astype(np.float32).reshape(PER_CORE_B, S, D)
            for i in range(NCORES)
        ],
        axis=0,
    )
    return out
